# revision 66
# baseline (speedup 1.0000x reference)
"""Trainium2 Bass kernel for the CMB power-spectrum emulator problem.

Math: a 4-layer MLP maps phi (512,2) -> diag (128 knots, 512 ch); a natural
cubic spline through the 128 knots is evaluated on a constant 256x256
isotropic-frequency grid, then exp(.)*NORM.

Two structural collapses, both input-independent:
 1. The spline is linear in the knot values, so the whole spline stage is
    one constant matrix E:  out = exp(E @ diag + ln NORM).
 2. The grid value wn_iso[i,j] depends only on (a,b) = sorted(|wn_i|,|wn_j|),
    an exact 8-fold dihedral symmetry: only 8385 of the 65536 grid points
    are distinct, and equal points produce bitwise-equal outputs. The device
    computes the 8385 unique points; the host replicates them with a
    constant gather.

Device work per core (unique-point sharding, 1056 points/core, 512 ch):
  MLP as two interleaved 256-wide chains (f32r matmuls on TensorE,
    relu+bias and the final bias-add on the otherwise-idle VectorE,
    keeping the ScalarE FIFO clear for the exp stream) -> diag (128, 512)
  per 128-channel group g: psum = diag_g.T @ ET_u  (TensorE, f32r)
                           stage = exp(psum+lnN)   (ScalarE LUT, ~2 ULP)
                           store (128, 1056) fp32  (SP HWDGE ring)
"""

import os

import numpy as np

B = 512
N_CORES = 8
N_UNIQ = 129 * 130 // 2       # 8385 distinct grid values
P_CORE = 1056                 # per-core unique points (8 x 1056 = 8448 padded)
P_PAD = N_CORES * P_CORE
NORM = 1.0 / 12661.0

MIN_PHI = np.array([50.0, 0.0075], np.float32)
DPHI = np.array([40.0, 0.0492], np.float32)
MU = np.array([70.0, 0.032], np.float32)
SIG = np.array([20.0, 0.025], np.float32)

# matmul dtype: "f32" (4 cyc/row, exact), "f32r" (1 cyc/row, ~19-bit mantissa)
MODE = os.environ.get("BASS_KERNEL_MODE", "f32r")

# packed-parameter column layout (partition dim x columns), matmul-dtype part
_PM_PHT = slice(0, 512)
_PM_W1 = slice(512, 612)
_PM_W2 = slice(612, 712)
_PM_W3 = slice(712, 812)
_PM_W4 = slice(812, 940)
PM_COLS = 940
PB_COLS = 5  # fp32 part: b1, b2, b3, b4, ln(NORM)

_CACHE = {}


def _spline_eval_matrix(wn_vals):
    """E (len(wn_vals), 128) fp32: natural-cubic-spline evaluation at wn_vals,
    linear in the 128 knot values (knots t_k = sqrt(2)*k in fp32)."""
    wn = (256.0 * np.fft.fftfreq(256, d=1.0)).reshape(256, 1)
    wn_iso = np.sqrt(wn**2 + wn.reshape(1, 256) ** 2)
    t32 = np.fft.fftshift(wn_iso).diagonal()[128:].astype(np.float32)  # (128,)

    n = 128
    t = t32.astype(np.float64)
    h = np.diff(t)
    A = np.diag(2.0 * (h[:-1] + h[1:])) + np.diag(h[1:-1], 1) + np.diag(h[1:-1], -1)
    D1 = np.zeros((n - 1, n))
    for i in range(n - 1):
        D1[i, i] = -1.0 / h[i]
        D1[i, i + 1] = 1.0 / h[i]
    D2 = 6.0 * (D1[1:] - D1[:-1])
    L = np.zeros((n, n))
    L[1:-1] = np.linalg.solve(A, D2)

    Sa = np.eye(n)[: n - 1]
    Sb = D1 - (h[:, None] / 6.0) * (2.0 * L[:-1] + L[1:])
    Sc = L[:-1] / 2.0
    Sd = (L[1:] - L[:-1]) / (6.0 * h[:, None])

    w32 = wn_vals.astype(np.float32)
    idx = np.clip(np.searchsorted(t32, w32, side="right") - 1, 0, n - 2)
    f = (w32 - t32[idx]).astype(np.float64)[:, None]
    E = Sa[idx] + f * (Sb[idx] + f * (Sc[idx] + f * Sd[idx]))
    return E.astype(np.float32)


def _build_constants():
    """ET_u (128, P_PAD) fp32 for the unique points, and IDX (65536,) int32
    mapping each full-grid point to its unique-point column."""
    k = np.arange(256)
    absw = np.minimum(k, 256 - k)  # |wn_i|, with |wn_0| = 0, |wn_128| = 128
    ai = np.minimum(absw[:, None], absw[None, :])
    bi = np.maximum(absw[:, None], absw[None, :])
    uid = (bi * (bi + 1)) // 2 + ai  # (256,256) in [0, N_UNIQ)

    bs = np.concatenate([np.full(b + 1, b) for b in range(129)])  # uid -> b
    as_ = np.concatenate([np.arange(b + 1) for b in range(129)])  # uid -> a
    wn_vals = np.sqrt((as_.astype(np.float64)) ** 2 + (bs.astype(np.float64)) ** 2)

    E = _spline_eval_matrix(wn_vals)  # (8385, 128)
    ET = np.zeros((128, P_PAD), np.float32)
    ET[:, :N_UNIQ] = E.T
    return np.ascontiguousarray(ET), uid.ravel().astype(np.int32)


def _build_program(mode):
    import concourse.bass as bass
    import concourse.bacc as bacc
    import concourse.mybir as mybir
    from concourse import tile

    f32 = mybir.dt.float32
    mm_dt = {"f32r": mybir.dt.float32r, "f32": f32, "mix": mybir.dt.float32r}[mode]
    main_dt = mybir.dt.bfloat16 if mode == "mix" else mm_dt
    nc = bacc.Bacc("TRN2", target_bir_lowering=False, debug=False)

    pm_d = nc.dram_tensor("pm", [128, PM_COLS], mm_dt, kind="ExternalInput")
    pb_d = nc.dram_tensor("pb", [128, PB_COLS], f32, kind="ExternalInput")
    et_d = nc.dram_tensor("et", [128, P_CORE], main_dt, kind="ExternalInput")
    out_d = nc.dram_tensor("out", [B, P_CORE], f32, kind="ExternalOutput")

    Relu = mybir.ActivationFunctionType.Relu
    Ident = mybir.ActivationFunctionType.Identity
    Exp = mybir.ActivationFunctionType.Exp

    N_GRP = 4
    SUB = 512  # matmul free chunk (PSUM bank)

    with tile.TileContext(nc) as tc:
        with (
            tc.tile_pool(name="const", bufs=1) as cpool,
            tc.tile_pool(name="mlp", bufs=2) as mpool,
            tc.tile_pool(name="stage", bufs=4) as spool,
            tc.tile_pool(name="psum", bufs=2, space=bass.MemorySpace.PSUM) as ppool,
            tc.tile_pool(name="mpsum", bufs=2, space=bass.MemorySpace.PSUM) as mps,
        ):
            # ---- loads on the idle SP ring: params first, then ET ----
            pm_t = cpool.tile([128, PM_COLS], mm_dt, tag="pm")
            nc.sync.dma_start(pm_t[:], pm_d[:])
            pb_t = cpool.tile([128, PB_COLS], f32, tag="pb")
            nc.sync.dma_start(pb_t[:], pb_d[:])
            et_t = cpool.tile([128, P_CORE], main_dt, tag="et")
            nc.sync.dma_start(et_t[:], et_d[:])

            pht = pm_t[0:2, _PM_PHT]
            w1 = pm_t[0:2, _PM_W1]
            w2 = pm_t[0:100, _PM_W2]
            w3 = pm_t[0:100, _PM_W3]
            w4 = pm_t[0:100, _PM_W4]
            b1 = pb_t[0:100, 0:1]
            b2 = pb_t[0:100, 1:2]
            b3 = pb_t[0:100, 2:3]
            b4 = pb_t[0:128, 3:4]
            lnb = pb_t[0:128, 4:5]


            # ---- MLP, two interleaved 256-wide chains (hides sem latency) ----
            HB = B // 2
            diag = mpool.tile([128, B], main_dt, tag="diag")
            hs = {}
            for lyr, (wt, bt, act, win, wout) in enumerate(
                [
                    (w1, b1, Relu, 2, 100),
                    (w2, b2, Relu, 100, 100),
                    (w3, b3, Relu, 100, 100),
                    (w4, b4, Ident, 100, 128),
                ]
            ):
                for c in range(2):
                    cs = slice(c * HB, (c + 1) * HB)
                    src = pht[:, cs] if lyr == 0 else hs[c][:]
                    ps = mps.tile([128, SUB], f32, tag="mps")
                    nc.tensor.matmul(ps[0:wout, 0:HB], wt, src)
                    if lyr < 3:
                        h = mpool.tile([100, HB], mm_dt, tag=f"h{lyr}{c}")
                        nc.vector.tensor_scalar(
                            h[:], ps[0:wout, 0:HB], bt, 0.0,
                            mybir.AluOpType.add, mybir.AluOpType.max,
                        )
                        hs[c] = h
                    else:
                        nc.vector.tensor_scalar(
                            diag[:, cs], ps[0:wout, 0:HB], bt, None,
                            mybir.AluOpType.add,
                        )

            # ---- main: out[g] = exp(diag_g.T @ ET_u + lnN), one store per g;
            # last group split in two so its store starts sooner ----
            for g in range(N_GRP):
                ps = ppool.tile([128, P_CORE], f32, tag="ps")
                for off in range(0, P_CORE, SUB):
                    w = min(SUB, P_CORE - off)
                    nc.tensor.matmul(
                        ps[:, off : off + w],
                        diag[:, g * 128 : (g + 1) * 128],
                        et_t[:, off : off + w],
                    )
                stage = spool.tile([128, P_CORE], f32, tag="stage")
                orow = out_d[g * 128 : (g + 1) * 128, :]
                if g < N_GRP - 1:
                    nc.scalar.activation(stage[:], ps[:], Exp, bias=lnb)
                    nc.sync.dma_start(orow, stage[:])
                else:
                    hp = 800  # late split: small final exp+store
                    nc.scalar.activation(stage[:, :hp], ps[:, :hp], Exp, bias=lnb)
                    nc.sync.dma_start(orow[:, :hp], stage[:, :hp])
                    nc.scalar.activation(stage[:, hp:], ps[:, hp:], Exp, bias=lnb)
                    nc.sync.dma_start(orow[:, hp:], stage[:, hp:])

    nc.compile()
    return nc


def _get_cached():
    key = ("nc", MODE)
    if key not in _CACHE:
        _CACHE[key] = _build_program(MODE)
    if "consts" not in _CACHE:
        _CACHE["consts"] = _build_constants()
    return (_CACHE[key],) + _CACHE["consts"]


def _make_in_maps(phi, W1, b1, W2, b2, W3, b3, W4, b4, ET):
    # fold the input normalization into the first layer
    scale = (DPHI / SIG).astype(np.float32)
    shift = ((MIN_PHI - MU) / SIG).astype(np.float32)
    W1f = (np.asarray(W1, np.float32) * scale[:, None]).astype(np.float32)
    b1f = (np.asarray(b1, np.float32) + shift @ np.asarray(W1, np.float32)).astype(
        np.float32
    )

    pm = np.zeros((128, PM_COLS), np.float32)
    pm[0:2, _PM_PHT] = np.asarray(phi, np.float32).T
    pm[0:2, _PM_W1] = W1f
    pm[0:100, _PM_W2] = np.asarray(W2, np.float32)
    pm[0:100, _PM_W3] = np.asarray(W3, np.float32)
    pm[0:100, _PM_W4] = np.asarray(W4, np.float32)
    pb = np.zeros((128, PB_COLS), np.float32)
    pb[0:100, 0] = np.asarray(b1f, np.float32)
    pb[0:100, 1] = np.asarray(b2, np.float32)
    pb[0:100, 2] = np.asarray(b3, np.float32)
    pb[0:128, 3] = np.asarray(b4, np.float32)
    pb[:, 4] = np.log(np.float64(NORM))

    common = {"pm": pm, "pb": pb}
    in_maps = []
    for c in range(N_CORES):
        m = dict(common)
        shard = np.ascontiguousarray(ET[:, c * P_CORE : (c + 1) * P_CORE])
        if MODE == "mix":
            import ml_dtypes

            shard = shard.astype(ml_dtypes.bfloat16)
        m["et"] = shard
        in_maps.append(m)
    return in_maps


def kernel(phi, W1, b1, W2, b2, W3, b3, W4, b4):
    from concourse.bass_utils import run_bass_kernel_spmd

    nc, ET, IDX = _get_cached()
    in_maps = _make_in_maps(phi, W1, b1, W2, b2, W3, b3, W4, b4, ET)
    res = run_bass_kernel_spmd(nc, in_maps, core_ids=list(range(N_CORES)))
    uniq = np.concatenate([r["out"] for r in res.results], axis=1)  # (512, 8448)
    full = np.take(uniq, IDX, axis=1)  # (512, 65536) constant-gather replication
    return np.ascontiguousarray(full.reshape(B, 256, 256))


# revision 67
# speedup vs baseline: 1.0181x; 1.0181x over previous
"""Trainium2 Bass kernel for the CMB power-spectrum emulator problem.

Math: a 4-layer MLP maps phi (512,2) -> diag (128 knots, 512 ch); a natural
cubic spline through the 128 knots is evaluated on a constant 256x256
isotropic-frequency grid, then exp(.)*NORM.

Two structural collapses, both input-independent:
 1. The spline is linear in the knot values, so the whole spline stage is
    one constant matrix E:  out = exp(E @ diag + ln NORM).
 2. The grid value wn_iso[i,j] depends only on (a,b) = sorted(|wn_i|,|wn_j|),
    an exact 8-fold dihedral symmetry: only 8385 of the 65536 grid points
    are distinct, and equal points produce bitwise-equal outputs. The device
    computes the 8385 unique points; the host replicates them with a
    constant gather.

Device work per core (unique-point sharding, 1056 points/core, 512 ch):
  MLP as two interleaved 256-wide chains (f32r matmuls on TensorE,
    relu+bias and the final bias-add on the otherwise-idle VectorE,
    keeping the ScalarE FIFO clear for the exp stream) -> diag (128, 512)
  per 128-channel group g: psum = diag_g.T @ ET_u  (TensorE, f32r)
                           stage = exp(psum+lnN)   (ScalarE LUT, ~2 ULP)
                           store (128, 1056) fp32  (SP HWDGE ring)
"""

import os

import numpy as np

B = 512
N_CORES = 8
N_UNIQ = 129 * 130 // 2       # 8385 distinct grid values
P_CORE = 1056                 # per-core unique points (8 x 1056 = 8448 padded)
P_PAD = N_CORES * P_CORE
NORM = 1.0 / 12661.0

MIN_PHI = np.array([50.0, 0.0075], np.float32)
DPHI = np.array([40.0, 0.0492], np.float32)
MU = np.array([70.0, 0.032], np.float32)
SIG = np.array([20.0, 0.025], np.float32)

# matmul dtype: "f32" (4 cyc/row, exact), "f32r" (1 cyc/row, ~19-bit mantissa)
MODE = os.environ.get("BASS_KERNEL_MODE", "f32r")

# packed parameters: ph (2 partitions: phiT|W1), pm (128p: W2|W3|W4)
PH_COLS = 612
PM_COLS = 328
PB_COLS = 5  # fp32 part: b1, b2, b3, b4, ln(NORM)

_CACHE = {}


def _spline_eval_matrix(wn_vals):
    """E (len(wn_vals), 128) fp32: natural-cubic-spline evaluation at wn_vals,
    linear in the 128 knot values (knots t_k = sqrt(2)*k in fp32)."""
    wn = (256.0 * np.fft.fftfreq(256, d=1.0)).reshape(256, 1)
    wn_iso = np.sqrt(wn**2 + wn.reshape(1, 256) ** 2)
    t32 = np.fft.fftshift(wn_iso).diagonal()[128:].astype(np.float32)  # (128,)

    n = 128
    t = t32.astype(np.float64)
    h = np.diff(t)
    A = np.diag(2.0 * (h[:-1] + h[1:])) + np.diag(h[1:-1], 1) + np.diag(h[1:-1], -1)
    D1 = np.zeros((n - 1, n))
    for i in range(n - 1):
        D1[i, i] = -1.0 / h[i]
        D1[i, i + 1] = 1.0 / h[i]
    D2 = 6.0 * (D1[1:] - D1[:-1])
    L = np.zeros((n, n))
    L[1:-1] = np.linalg.solve(A, D2)

    Sa = np.eye(n)[: n - 1]
    Sb = D1 - (h[:, None] / 6.0) * (2.0 * L[:-1] + L[1:])
    Sc = L[:-1] / 2.0
    Sd = (L[1:] - L[:-1]) / (6.0 * h[:, None])

    w32 = wn_vals.astype(np.float32)
    idx = np.clip(np.searchsorted(t32, w32, side="right") - 1, 0, n - 2)
    f = (w32 - t32[idx]).astype(np.float64)[:, None]
    E = Sa[idx] + f * (Sb[idx] + f * (Sc[idx] + f * Sd[idx]))
    return E.astype(np.float32)


def _build_constants():
    """ET_u (128, P_PAD) fp32 for the unique points, and IDX (65536,) int32
    mapping each full-grid point to its unique-point column."""
    k = np.arange(256)
    absw = np.minimum(k, 256 - k)  # |wn_i|, with |wn_0| = 0, |wn_128| = 128
    ai = np.minimum(absw[:, None], absw[None, :])
    bi = np.maximum(absw[:, None], absw[None, :])
    uid = (bi * (bi + 1)) // 2 + ai  # (256,256) in [0, N_UNIQ)

    bs = np.concatenate([np.full(b + 1, b) for b in range(129)])  # uid -> b
    as_ = np.concatenate([np.arange(b + 1) for b in range(129)])  # uid -> a
    wn_vals = np.sqrt((as_.astype(np.float64)) ** 2 + (bs.astype(np.float64)) ** 2)

    E = _spline_eval_matrix(wn_vals)  # (8385, 128)
    ET = np.zeros((128, P_PAD), np.float32)
    ET[:, :N_UNIQ] = E.T
    return np.ascontiguousarray(ET), uid.ravel().astype(np.int32)


def _build_program(mode):
    import concourse.bass as bass
    import concourse.bacc as bacc
    import concourse.mybir as mybir
    from concourse import tile

    f32 = mybir.dt.float32
    mm_dt = {"f32r": mybir.dt.float32r, "f32": f32, "mix": mybir.dt.float32r}[mode]
    main_dt = mybir.dt.bfloat16 if mode == "mix" else mm_dt
    nc = bacc.Bacc("TRN2", target_bir_lowering=False, debug=False)

    ph_d = nc.dram_tensor("ph", [2, PH_COLS], mm_dt, kind="ExternalInput")
    pm_d = nc.dram_tensor("pm", [128, PM_COLS], mm_dt, kind="ExternalInput")
    pb_d = nc.dram_tensor("pb", [128, PB_COLS], f32, kind="ExternalInput")
    et_d = nc.dram_tensor("et", [128, P_CORE], main_dt, kind="ExternalInput")
    out_d = nc.dram_tensor("out", [B, P_CORE], f32, kind="ExternalOutput")

    Relu = mybir.ActivationFunctionType.Relu
    Ident = mybir.ActivationFunctionType.Identity
    Exp = mybir.ActivationFunctionType.Exp

    N_GRP = 4
    SUB = 512  # matmul free chunk (PSUM bank)

    with tile.TileContext(nc) as tc:
        with (
            tc.tile_pool(name="const", bufs=1) as cpool,
            tc.tile_pool(name="mlp", bufs=2) as mpool,
            tc.tile_pool(name="stage", bufs=4) as spool,
            tc.tile_pool(name="psum", bufs=2, space=bass.MemorySpace.PSUM) as ppool,
            tc.tile_pool(name="mpsum", bufs=2, space=bass.MemorySpace.PSUM) as mps,
        ):
            # ---- loads on the idle SP ring: params first, then ET ----
            ph_t = cpool.tile([2, PH_COLS], mm_dt, tag="ph")
            nc.sync.dma_start(ph_t[:], ph_d[:])
            pm_t = cpool.tile([128, PM_COLS], mm_dt, tag="pm")
            nc.sync.dma_start(pm_t[:], pm_d[:])
            pb_t = cpool.tile([128, PB_COLS], f32, tag="pb")
            nc.sync.dma_start(pb_t[:], pb_d[:])
            et_t = cpool.tile([128, P_CORE], main_dt, tag="et")
            nc.sync.dma_start(et_t[:], et_d[:])

            pht = ph_t[0:2, 0:512]
            w1 = ph_t[0:2, 512:612]
            w2 = pm_t[0:100, 0:100]
            w3 = pm_t[0:100, 100:200]
            w4 = pm_t[0:100, 200:328]
            b1 = pb_t[0:100, 0:1]
            b2 = pb_t[0:100, 1:2]
            b3 = pb_t[0:100, 2:3]
            b4 = pb_t[0:128, 3:4]
            lnb = pb_t[0:128, 4:5]


            # ---- MLP, two interleaved 256-wide chains (hides sem latency) ----
            HB = B // 2
            diag = mpool.tile([128, B], main_dt, tag="diag")
            hs = {}
            for lyr, (wt, bt, act, win, wout) in enumerate(
                [
                    (w1, b1, Relu, 2, 100),
                    (w2, b2, Relu, 100, 100),
                    (w3, b3, Relu, 100, 100),
                    (w4, b4, Ident, 100, 128),
                ]
            ):
                for c in range(2):
                    cs = slice(c * HB, (c + 1) * HB)
                    src = pht[:, cs] if lyr == 0 else hs[c][:]
                    ps = mps.tile([128, SUB], f32, tag="mps")
                    nc.tensor.matmul(ps[0:wout, 0:HB], wt, src)
                    if lyr < 3:
                        h = mpool.tile([100, HB], mm_dt, tag=f"h{lyr}{c}")
                        nc.vector.tensor_scalar(
                            h[:], ps[0:wout, 0:HB], bt, 0.0,
                            mybir.AluOpType.add, mybir.AluOpType.max,
                        )
                        hs[c] = h
                    else:
                        nc.vector.tensor_scalar(
                            diag[:, cs], ps[0:wout, 0:HB], bt, None,
                            mybir.AluOpType.add,
                        )

            # ---- main: out[g] = exp(diag_g.T @ ET_u + lnN), one store per g;
            # last group split in two so its store starts sooner ----
            for g in range(N_GRP):
                ps = ppool.tile([128, P_CORE], f32, tag="ps")
                for off in range(0, P_CORE, SUB):
                    w = min(SUB, P_CORE - off)
                    nc.tensor.matmul(
                        ps[:, off : off + w],
                        diag[:, g * 128 : (g + 1) * 128],
                        et_t[:, off : off + w],
                    )
                stage = spool.tile([128, P_CORE], f32, tag="stage")
                orow = out_d[g * 128 : (g + 1) * 128, :]
                if g < N_GRP - 1:
                    nc.scalar.activation(stage[:], ps[:], Exp, bias=lnb)
                    nc.sync.dma_start(orow, stage[:])
                else:
                    hp = 800  # late split: small final exp+store
                    nc.scalar.activation(stage[:, :hp], ps[:, :hp], Exp, bias=lnb)
                    nc.sync.dma_start(orow[:, :hp], stage[:, :hp])
                    nc.scalar.activation(stage[:, hp:], ps[:, hp:], Exp, bias=lnb)
                    nc.sync.dma_start(orow[:, hp:], stage[:, hp:])

    nc.compile()
    return nc


def _get_cached():
    key = ("nc", MODE)
    if key not in _CACHE:
        _CACHE[key] = _build_program(MODE)
    if "consts" not in _CACHE:
        _CACHE["consts"] = _build_constants()
    return (_CACHE[key],) + _CACHE["consts"]


def _make_in_maps(phi, W1, b1, W2, b2, W3, b3, W4, b4, ET):
    # fold the input normalization into the first layer
    scale = (DPHI / SIG).astype(np.float32)
    shift = ((MIN_PHI - MU) / SIG).astype(np.float32)
    W1f = (np.asarray(W1, np.float32) * scale[:, None]).astype(np.float32)
    b1f = (np.asarray(b1, np.float32) + shift @ np.asarray(W1, np.float32)).astype(
        np.float32
    )

    ph = np.zeros((2, PH_COLS), np.float32)
    ph[:, 0:512] = np.asarray(phi, np.float32).T
    ph[:, 512:612] = W1f
    pm = np.zeros((128, PM_COLS), np.float32)
    pm[0:100, 0:100] = np.asarray(W2, np.float32)
    pm[0:100, 100:200] = np.asarray(W3, np.float32)
    pm[0:100, 200:328] = np.asarray(W4, np.float32)
    pb = np.zeros((128, PB_COLS), np.float32)
    pb[0:100, 0] = np.asarray(b1f, np.float32)
    pb[0:100, 1] = np.asarray(b2, np.float32)
    pb[0:100, 2] = np.asarray(b3, np.float32)
    pb[0:128, 3] = np.asarray(b4, np.float32)
    pb[:, 4] = np.log(np.float64(NORM))

    common = {"ph": ph, "pm": pm, "pb": pb}
    in_maps = []
    for c in range(N_CORES):
        m = dict(common)
        shard = np.ascontiguousarray(ET[:, c * P_CORE : (c + 1) * P_CORE])
        if MODE == "mix":
            import ml_dtypes

            shard = shard.astype(ml_dtypes.bfloat16)
        m["et"] = shard
        in_maps.append(m)
    return in_maps


def kernel(phi, W1, b1, W2, b2, W3, b3, W4, b4):
    from concourse.bass_utils import run_bass_kernel_spmd

    nc, ET, IDX = _get_cached()
    in_maps = _make_in_maps(phi, W1, b1, W2, b2, W3, b3, W4, b4, ET)
    res = run_bass_kernel_spmd(nc, in_maps, core_ids=list(range(N_CORES)))
    uniq = np.concatenate([r["out"] for r in res.results], axis=1)  # (512, 8448)
    full = np.take(uniq, IDX, axis=1)  # (512, 65536) constant-gather replication
    return np.ascontiguousarray(full.reshape(B, 256, 256))
